# revision 15
# baseline (speedup 1.0000x reference)
"""Trainium2 Bass kernel for nn_Block_68753836474893 (dual-attention block).

Sharding: 8 cores = 2 batches x 4 query-chunks of 576 tokens. Each core
redundantly computes the full-batch prefix (LN1, pos dwconv, K/V for both
attention branches) and exclusively computes its 576-token slice of the
output. No cross-core communication; host concatenates slices.

On-device layout is feature-major: [channel partitions, token free].
Per-token LN stats are reduced over partitions with ones-matmuls, bounced
through DRAM, and re-broadcast with 0-stride-partition DMA reads.
"""
import sys

sys.path.insert(0, "/opt/trn_rl_repo")

import contextlib
import itertools
import os

KSTAGE = int(os.environ.get("KSTAGE", "4"))

import numpy as np
import concourse.bass as bass
import concourse.tile as tile
from concourse import mybir, bacc, bass_utils
from concourse.bass import ds

B, HH, WW, C = 2, 48, 48, 256
N = HH * WW            # 2304
NH, DH = 8, 32
HID = 4 * C            # 1024
EPS = 1e-6
Q = 576                # query tokens per core
MARG = 96              # 2 grid rows of zero margin each side of the token axis
EXT = MARG + N + MARG  # 2496
WIN = 768              # 16 grid rows: chunk + 2-row halo each side
SCALE = DH ** -0.5

F32 = mybir.dt.float32
BF16 = mybir.dt.bfloat16
U32 = mybir.dt.uint32
AL = mybir.AluOpType
AF = mybir.ActivationFunctionType

CV_N1G, CV_N1B, CV_N2G, CV_N2B, CV_POSB, CV_LEPB, CV_PROJB, CV_P2B, CV_GB = range(9)


def _chunks(total, step):
    return [(s, min(step, total - s)) for s in range(0, total, step)]


def _build_kernel():
    nc = bacc.Bacc("TRN2", target_bir_lowering=False, debug=False,
                   enable_asserts=True, num_devices=8)
    dd = {}
    for name, shape, dt in [
        ("xt", [C, N], BF16), ("qoff", [1, 1], U32),
        ("qkvw", [C, 3 * C], BF16), ("projw", [C, C], BF16),
        ("p1w", [C, HID], BF16), ("p2w", [HID, C], BF16),
        ("gw", [HID, C], BF16), ("posw", [C, 9], F32),
        ("lepw", [C, 25], F32), ("cvec", [C, 12], F32),
        ("p1b2", [128, 8], F32), ("iden", [128, 128], F32),
        ("mvec", [128, 4], F32),
    ]:
        dd[name] = nc.dram_tensor(name, shape, dt, kind="ExternalInput").ap()
    dd["y"] = nc.dram_tensor("y", [Q, C], F32, kind="ExternalOutput").ap()

    with tile.TileContext(nc) as tc:
        _body(nc, tc, dd)
    nc.compile()
    return nc


def _body(nc, tc, dd):
    stack = contextlib.ExitStack()
    cnt = itertools.count()

    class _P:
        def __init__(self, p):
            self._p = p

        def tile(self, *a, **k):
            if "name" not in k:
                k["name"] = f"{k.get('tag', 't')}_{next(cnt)}"
            if "tag" not in k:
                k["tag"] = k["name"]
            return self._p.tile(*a, **k)

    def pool(name, bufs, **kw):
        return _P(stack.enter_context(tc.tile_pool(name=name, bufs=bufs, **kw)))

    p_x = pool("x", 1)        # xt bf16; tags x0/x1 reused by h2/g2 (f32 Q)
    p_big = pool("big", 2)    # [128,N] bf16 scratch: LN squares, conv accs
    p_ext = pool("ext", 1)    # [128,EXT] bf16 h_ext / lnh_ext
    p_w = pool("w", 1)        # weights
    p_kt = pool("kt", 2)      # [128,N] bf16 K^T
    p_v = pool("v", 18)       # [128,8,33] bf16 V(+ones)
    p_qt = pool("qt", 4)      # [128,Q] bf16 Q^T
    p_attn = pool("attn", 3)  # [128,2,288] bf16 exp tiles
    p_pad = pool("pad", 1)    # bf16 conv padded buffers
    p_c576 = pool("c576", 6)  # [128,Q] transients (lep/attout bf16, tt/g2 f32)
    p_per = pool("per", 1)    # persistent [128,Q] f32: yb/x1/x2/t2/outT/osb
    p_win = pool("win", 1)    # [128,WIN] bf16 windows, 4 tags
    p_bc = pool("bc", 2)      # [128,512] f32 broadcast chunks
    p_sm = pool("sm", 2)      # small stat tiles
    p_h1 = pool("h1", 8)      # [128,Q] bf16 mlp hidden
    p_x2b = pool("x2b", 1)    # [128,Q] bf16 x2 copy, 2 tags
    p_dr = pool("dr", 2, space="DRAM")
    ps_sc = pool("ps_sc", 2, space="PSUM")   # [128,2,512] scores
    ps_av = pool("ps_av", 2, space="PSUM")   # [128,288] AV accumulators
    ps_acc = pool("ps_acc", 2, space="PSUM")  # [128,512] general

    # ---- load inputs ----
    xt = [p_x.tile([128, N], BF16, tag=f"x{ct}") for ct in range(2)]
    qkvw = [p_w.tile([128, 3 * C], BF16, tag=f"qkvw{ct}") for ct in range(2)]
    projw = [p_w.tile([128, C], BF16, tag=f"projw{ct}") for ct in range(2)]
    p1w = [p_w.tile([128, HID], BF16, tag=f"p1w{ct}") for ct in range(2)]
    posw = [p_w.tile([128, 9], F32, tag=f"posw{ct}") for ct in range(2)]
    lepw = [p_w.tile([128, 25], F32, tag=f"lepw{ct}") for ct in range(2)]
    cvec = [p_w.tile([128, 12], F32, tag=f"cvec{ct}") for ct in range(2)]
    for ct in range(2):
        sl = slice(128 * ct, 128 * (ct + 1))
        nc.sync.dma_start(xt[ct][:], dd["xt"][sl, :])
        nc.sync.dma_start(qkvw[ct][:], dd["qkvw"][sl, :])
        nc.sync.dma_start(projw[ct][:], dd["projw"][sl, :])
        nc.sync.dma_start(p1w[ct][:], dd["p1w"][sl, :])
        nc.sync.dma_start(posw[ct][:], dd["posw"][sl, :])
        nc.sync.dma_start(lepw[ct][:], dd["lepw"][sl, :])
        nc.sync.dma_start(cvec[ct][:], dd["cvec"][sl, :])
    p2w = [p_w.tile([128, C], BF16, tag=f"p2w{h}") for h in range(8)]
    gw = [p_w.tile([128, C], BF16, tag=f"gw{h}") for h in range(8)]
    for h in range(8):
        nc.sync.dma_start(p2w[h][:], dd["p2w"][128 * h:128 * (h + 1), :])
        nc.sync.dma_start(gw[h][:], dd["gw"][128 * h:128 * (h + 1), :])
    p1b = p_w.tile([128, 8], F32, tag="p1b")
    nc.sync.dma_start(p1b[:], dd["p1b2"][:, :])
    iden = p_w.tile([128, 128], F32, tag="iden")
    nc.sync.dma_start(iden[:], dd["iden"][:, :])
    ones_b = p_w.tile([128, 1], BF16, tag="ones_b")
    nc.vector.memset(ones_b[:], 1.0)
    ones_f = p_w.tile([128, 1], F32, tag="ones_f")
    nc.vector.memset(ones_f[:], 1.0)
    epst = p_w.tile([128, 1], F32, tag="epst")
    nc.vector.memset(epst[:], EPS)
    mvec = p_w.tile([128, 4], F32, tag="mvec")
    nc.sync.dma_start(mvec[:], dd["mvec"][:, :])

    def blend_window(dst, ext):
        for qc in range(4):
            sl = ext[:, Q * qc:Q * qc + WIN]
            if qc == 0:
                nc.vector.tensor_scalar(dst[:], sl, mvec[:, 0:1], None, AL.mult)
            else:
                nc.vector.scalar_tensor_tensor(dst[:], sl, mvec[:, qc:qc + 1],
                                               dst[:], AL.mult, AL.add)

    def cv(ct, col):
        return cvec[ct][:, col:col + 1]

    def bail():
        for (s, w) in _chunks(Q, 128):
            osb = p_c576.tile([128, C], F32, tag="c576f")
            nc.vector.memset(osb[:], 0.0)
            nc.sync.dma_start(dd["y"][s:s + w, :], osb[0:w, :])
        stack.close()

    def bcast_ap(dr_ap, off, w):
        """DRAM AP read broadcast across 128 partitions."""
        return bass.AP(tensor=dr_ap.tensor, offset=dr_ap.offset + off,
                       ap=[[0, 128], [1, w]])

    def layernorm(src_tiles, out_tiles, width, gcol, bcol, st_shape, ones_t,
                  sq_dt):
        """out = (src - mu) * rsqrt(var+eps) * g + b per token (over C)."""
        sq = [p_big.tile([128, N], sq_dt, tag="big") for _ in range(2)]
        for ct in range(2):
            nc.vector.tensor_tensor(sq[ct][:, :width], src_tiles[ct],
                                    src_tiles[ct], AL.mult)
        dr_s = p_dr.tile([width], F32, tag="dr_s")
        dr_q = p_dr.tile([width], F32, tag="dr_q")
        for (dst, srcs) in ((dr_s, src_tiles),
                            (dr_q, [sq[0][:, :width], sq[1][:, :width]])):
            for (s, w) in _chunks(width, 512):
                ps = ps_acc.tile([128, 512], F32, tag="acc")
                for ct in range(2):
                    nc.tensor.matmul(ps[0:1, :w], ones_t[:],
                                     srcs[ct][:, s:s + w],
                                     start=(ct == 0), stop=(ct == 1))
                b512 = p_sm.tile([1, 512], F32, tag="b512")
                nc.vector.tensor_copy(out=b512[0:1, :w], in_=ps[0:1, :w])
                nc.sync.dma_start(dst[s:s + w], b512[0:1, :w])
        pp, ff = st_shape
        st_s = p_sm.tile([pp, ff], F32, tag="st_s")
        st_q = p_sm.tile([pp, ff], F32, tag="st_q")
        nc.sync.dma_start(st_s[:], dr_s.rearrange("(p f) -> p f", p=pp))
        nc.sync.dma_start(st_q[:], dr_q.rearrange("(p f) -> p f", p=pp))
        nc.vector.tensor_scalar(st_s[:], st_s[:], 1.0 / C, None, AL.mult)
        nc.vector.tensor_scalar(st_q[:], st_q[:], 1.0 / C, None, AL.mult)
        musq = p_sm.tile([pp, ff], F32, tag="musq")
        nc.vector.tensor_tensor(musq[:], st_s[:], st_s[:], AL.mult)
        nc.vector.tensor_tensor(st_q[:], st_q[:], musq[:], AL.subtract)
        nc.scalar.activation(st_q[:], st_q[:], AF.Sqrt, bias=epst[0:pp, 0:1])
        nc.vector.reciprocal(st_q[:], st_q[:])                       # r
        nc.vector.tensor_tensor(st_s[:], st_q[:], st_s[:], AL.mult)  # r*mu
        dr_r = p_dr.tile([width], F32, tag="dr_r")
        dr_m = p_dr.tile([width], F32, tag="dr_m")
        nc.sync.dma_start(dr_r.rearrange("(p f) -> p f", p=pp), st_q[:])
        nc.sync.dma_start(dr_m.rearrange("(p f) -> p f", p=pp), st_s[:])
        for (s, w) in _chunks(width, 512):
            rb = p_bc.tile([128, 512], F32, tag="rb")
            mb = p_bc.tile([128, 512], F32, tag="mb")
            nc.gpsimd.dma_start(rb[:, :w], bcast_ap(dr_r, s, w))
            nc.gpsimd.dma_start(mb[:, :w], bcast_ap(dr_m, s, w))
            for ct in range(2):
                t = p_bc.tile([128, 512], F32, tag="t")
                nc.vector.tensor_tensor(t[:, :w], src_tiles[ct][:, s:s + w],
                                        rb[:, :w], AL.mult)
                nc.vector.tensor_tensor(t[:, :w], t[:, :w], mb[:, :w],
                                        AL.subtract)
                nc.vector.tensor_scalar(out_tiles[ct][:, s:s + w], t[:, :w],
                                        cv(ct, gcol), cv(ct, bcol),
                                        AL.mult, AL.add)

    # ---- LN1 into h_ext interior ----
    if KSTAGE < 1:
        bail()
        return
    h_ext = [p_ext.tile([128, EXT], BF16, tag=f"hext{ct}") for ct in range(2)]
    lnh_ext = [p_ext.tile([128, EXT], BF16, tag=f"lnhext{ct}")
               for ct in range(2)]
    for ct in range(2):
        for e in (h_ext, lnh_ext):
            nc.vector.memset(e[ct][:, 0:MARG], 0.0)
            nc.vector.memset(e[ct][:, MARG + N:EXT], 0.0)
    h_int = [h_ext[ct][:, MARG:MARG + N] for ct in range(2)]
    lnh_int = [lnh_ext[ct][:, MARG:MARG + N] for ct in range(2)]
    layernorm([xt[0][:], xt[1][:]], h_int, N, CV_N1G, CV_N1B, (128, 18),
              ones_b, BF16)

    # ---- pos dwconv 3x3: h = ln1 + conv(ln1) + pos_b ----
    for ct in range(2):
        pad3 = p_pad.tile([128, 50, 50], BF16, tag="pad")
        nc.vector.memset(pad3[:], 0.0)
        nc.vector.tensor_copy(
            out=pad3[:, 1:49, 1:49],
            in_=h_int[ct].rearrange("p (r w) -> p r w", r=48))
        acc = p_big.tile([128, N], BF16, tag="big")
        acc3 = acc.rearrange("p (r w) -> p r w", r=48)
        for t9 in range(9):
            di, dj = t9 // 3, t9 % 3
            src = pad3[:, di:di + 48, dj:dj + 48]
            wsc = posw[ct][:, t9:t9 + 1]
            if t9 == 0:
                nc.vector.tensor_scalar(acc3, src, wsc, None, AL.mult)
            else:
                nc.vector.scalar_tensor_tensor(acc3, src, wsc, acc3,
                                               AL.mult, AL.add)
        nc.vector.scalar_tensor_tensor(h_int[ct], acc[:], cv(ct, CV_POSB),
                                       h_int[ct], AL.add, AL.add)

    h_win = [p_win.tile([128, WIN], BF16, tag=f"hwin{ct}") for ct in range(2)]
    for ct in range(2):
        blend_window(h_win[ct], h_ext[ct])

    def attn_branch(xa, xa_win, br):
        kt = [p_kt.tile([128, N], BF16, tag="kt") for _ in range(2)]
        for g in range(2):
            for (s, w) in _chunks(N, 512):
                ps = ps_acc.tile([128, 512], F32, tag="acc")
                for ct in range(2):
                    nc.tensor.matmul(
                        ps[:, :w], qkvw[ct][:, C + 128 * g:C + 128 * (g + 1)],
                        xa[ct][:, s:s + w], start=(ct == 0), stop=(ct == 1))
                nc.any.tensor_copy(out=kt[g][:, s:s + w], in_=ps[:, :w])
        vt = []
        for tk in range(18):
            ps = ps_acc.tile([128, 512], F32, tag="acc")
            for ct in range(2):
                nc.tensor.matmul(ps[:, :C], xa[ct][:, 128 * tk:128 * (tk + 1)],
                                 qkvw[ct][:, 2 * C:3 * C],
                                 start=(ct == 0), stop=(ct == 1))
            v = p_v.tile([128, 8, 33], BF16, tag="v")
            nc.any.tensor_copy(out=v[:, :, 0:32],
                               in_=ps[:, :C].rearrange("p (h d) -> p h d", h=8))
            nc.vector.memset(v[:, :, 32:33], 1.0)
            vt.append(v)
        qt = [p_qt.tile([128, Q], BF16, tag="qt") for _ in range(2)]
        for g in range(2):
            for (s, w) in _chunks(Q, 288):
                ps = ps_acc.tile([128, 512], F32, tag="acc")
                for ct in range(2):
                    nc.tensor.matmul(
                        ps[:, :w], qkvw[ct][:, 128 * g:128 * (g + 1)],
                        xa_win[ct][:, MARG + s:MARG + s + w],
                        start=(ct == 0), stop=(ct == 1))
                nc.any.tensor_copy(out=qt[g][:, s:s + w], in_=ps[:, :w])
        lep = [p_c576.tile([128, Q], BF16, tag="c576b") for _ in range(2)]
        for ct in range(2):
            pad5 = p_pad.tile([128, 16, 52], BF16, tag="pad")
            nc.vector.memset(pad5[:], 0.0)
            nc.vector.tensor_copy(
                out=pad5[:, :, 2:50],
                in_=xa_win[ct].rearrange("p (r w) -> p r w", r=16))
            lep3 = lep[ct].rearrange("p (r w) -> p r w", r=12)
            for t25 in range(25):
                di, dj = t25 // 5, t25 % 5
                src = pad5[:, di:di + 12, dj:dj + 48]
                wsc = lepw[ct][:, t25:t25 + 1]
                if t25 == 0:
                    nc.vector.tensor_scalar(lep3, src, wsc, None, AL.mult)
                else:
                    nc.vector.scalar_tensor_tensor(lep3, src, wsc, lep3,
                                                   AL.mult, AL.add)
        attout = [p_c576.tile([128, Q], BF16, tag="c576b") for _ in range(2)]
        sumsg = [p_bc.tile([128, Q], F32, tag="sumsg") for _ in range(2)]
        for g in range(2):
            for pr in range(2):
                for (s, w) in _chunks(Q, 288):
                    avh = [ps_av.tile([128, 288], F32, tag="av")
                           for _ in range(2)]
                    for kc in range(18):
                        scp = ps_sc.tile([128, 2, 512], F32, tag="sc")
                        for r2 in range(2):
                            r = 2 * pr + r2
                            nc.tensor.matmul(
                                scp[:, r2, 0:w],
                                kt[g][32 * r:32 * (r + 1),
                                      128 * kc:128 * (kc + 1)],
                                qt[g][32 * r:32 * (r + 1), s:s + w],
                                tile_position=(32 * r, 0))
                        at = p_attn.tile([128, 2, 288], BF16, tag="attn")
                        nc.scalar.activation(at[:, :, 0:w], scp[:, :, 0:w],
                                             AF.Exp, scale=SCALE)
                        for r2 in range(2):
                            h = 4 * g + 2 * pr + r2
                            nc.tensor.matmul(avh[r2][0:33, :w],
                                             vt[kc][:, h, :], at[:, r2, 0:w],
                                             start=(kc == 0), stop=(kc == 17))
                    for r2 in range(2):
                        r = 2 * pr + r2
                        nc.vector.tensor_copy(
                            out=attout[g][32 * r:32 * (r + 1), s:s + w],
                            in_=avh[r2][0:32, :w])
                        nc.vector.tensor_copy(
                            out=sumsg[g][32 * r:32 * r + 1, s:s + w],
                            in_=avh[r2][32:33, :w])
        for g in range(2):
            dr_sg = p_dr.tile([4 * Q], F32, tag="dr_sg")
            for r in range(4):
                nc.sync.dma_start(dr_sg[r * Q:(r + 1) * Q],
                                  sumsg[g][32 * r:32 * r + 1, :])
            rbq = p_bc.tile([128, Q], F32, tag="rbq")
            for r in range(4):
                nc.gpsimd.dma_start(
                    rbq[32 * r:32 * (r + 1), :],
                    bass.AP(tensor=dr_sg.tensor, offset=dr_sg.offset + r * Q,
                            ap=[[0, 32], [1, Q]]))
            nc.vector.reciprocal(rbq[:], rbq[:])
            nc.vector.tensor_tensor(attout[g][:], attout[g][:], rbq[:],
                                    AL.mult)
            nc.vector.scalar_tensor_tensor(attout[g][:], lep[g][:],
                                           cv(g, CV_LEPB), attout[g][:],
                                           AL.add, AL.add)
        yb = [p_per.tile([128, Q], F32, tag=f"yb{br}_{og}") for og in range(2)]
        for og in range(2):
            for (s, w) in _chunks(Q, 288):
                ps = ps_acc.tile([128, 512], F32, tag="acc")
                for ct in range(2):
                    nc.tensor.matmul(ps[:, :w],
                                     projw[ct][:, 128 * og:128 * (og + 1)],
                                     attout[ct][:, s:s + w],
                                     start=(ct == 0), stop=(ct == 1))
                nc.vector.tensor_scalar(yb[og][:, s:s + w], ps[:, :w],
                                        cv(og, CV_PROJB), None, AL.add)
        return yb

    if KSTAGE < 2:
        bail()
        return
    yb2 = attn_branch(h_int, h_win, 2)
    if KSTAGE < 3:
        bail()
        return
    layernorm(h_int, lnh_int, N, CV_N1G, CV_N1B, (128, 18), ones_b, BF16)
    lnh_win = [p_win.tile([128, WIN], BF16, tag=f"lwin{ct}") for ct in range(2)]
    for ct in range(2):
        blend_window(lnh_win[ct], lnh_ext[ct])
    yb1 = attn_branch(lnh_int, lnh_win, 1)

    if KSTAGE < 4:
        bail()
        return
    hc = [h_win[ct][:, MARG:MARG + Q] for ct in range(2)]
    x1 = [p_per.tile([128, Q], F32, tag=f"x1_{ct}") for ct in range(2)]
    tt = [p_c576.tile([128, Q], F32, tag="c576f") for _ in range(2)]
    x2 = [p_per.tile([128, Q], F32, tag=f"x2_{ct}") for ct in range(2)]
    for ct in range(2):
        nc.vector.tensor_tensor(x1[ct][:], hc[ct], yb1[ct][:], AL.add)
        nc.vector.tensor_tensor(tt[ct][:], hc[ct], yb2[ct][:], AL.add)
    layernorm([tt[0][:], tt[1][:]], [x2[0][:], x2[1][:]], Q,
              CV_N1G, CV_N1B, (64, 9), ones_f, F32)
    x2b = [p_x2b.tile([128, Q], BF16, tag=f"x2b{ct}") for ct in range(2)]
    for ct in range(2):
        nc.vector.tensor_tensor(x2[ct][:], x2[ct][:], x1[ct][:], AL.add)
        nc.vector.tensor_copy(out=x2b[ct][:], in_=x2[ct][:])

    # ---- gated MLP ----
    h1 = [p_h1.tile([128, Q], BF16, tag="h1") for _ in range(8)]
    for hg in range(8):
        for (s, w) in _chunks(Q, 288):
            ps = ps_acc.tile([128, 512], F32, tag="acc")
            for ct in range(2):
                nc.tensor.matmul(ps[:, :w],
                                 p1w[ct][:, 128 * hg:128 * (hg + 1)],
                                 x2b[ct][:, s:s + w],
                                 start=(ct == 0), stop=(ct == 1))
            nc.scalar.activation(h1[hg][:, s:s + w], ps[:, :w], AF.Gelu,
                                 bias=p1b[:, hg:hg + 1], scale=1.0)
    h2 = [p_x.tile([128, Q], F32, tag=f"x{og}") for og in range(2)]
    g2 = [p_c576.tile([128, Q], F32, tag="c576f") for _ in range(2)]
    for og in range(2):
        for (wmat, dst, bcol) in ((p2w, h2, CV_P2B), (gw, g2, CV_GB)):
            for (s, w) in _chunks(Q, 288):
                ps = ps_acc.tile([128, 512], F32, tag="acc")
                for hg in range(8):
                    nc.tensor.matmul(ps[:, :w],
                                     wmat[hg][:, 128 * og:128 * (og + 1)],
                                     h1[hg][:, s:s + w],
                                     start=(hg == 0), stop=(hg == 7))
                nc.vector.tensor_scalar(dst[og][:, s:s + w], ps[:, :w],
                                        cv(og, bcol), None, AL.add)
    t2 = [p_per.tile([128, Q], F32, tag=f"t2_{ct}") for ct in range(2)]
    for ct in range(2):
        nc.vector.tensor_tensor(g2[ct][:], h2[ct][:], g2[ct][:], AL.mult)
        nc.vector.tensor_tensor(t2[ct][:], x2[ct][:], g2[ct][:], AL.add)

    outT = [p_per.tile([128, Q], F32, tag=f"outT{ct}") for ct in range(2)]
    layernorm([t2[0][:], t2[1][:]], [outT[0][:], outT[1][:]], Q,
              CV_N2G, CV_N2B, (64, 9), ones_f, F32)

    for (s, w) in _chunks(Q, 128):
        osb = p_c576.tile([128, C], F32, tag="c576f")
        for ct in range(2):
            ps = ps_acc.tile([128, 512], F32, tag="acc")
            nc.tensor.transpose(ps[0:w, 0:128], outT[ct][:, s:s + w], iden[:])
            nc.vector.tensor_copy(out=osb[0:w, 128 * ct:128 * (ct + 1)],
                                  in_=ps[0:w, 0:128])
        nc.sync.dma_start(dd["y"][s:s + w, :], osb[0:w, :])
    stack.close()


_NC_CACHE = {}


def _get_nc():
    if "nc" not in _NC_CACHE:
        _NC_CACHE["nc"] = _build_kernel()
    return _NC_CACHE["nc"]


def _make_inmaps(inputs):
    import ml_dtypes
    bf = ml_dtypes.bfloat16
    x = np.asarray(inputs["x"], np.float32)
    qkv_w = np.asarray(inputs["qkv_w"], np.float32).astype(bf)
    proj_w = np.asarray(inputs["proj_w"], np.float32).astype(bf)
    p1_w = np.asarray(inputs["p1_w"], np.float32).astype(bf)
    p2_w = np.asarray(inputs["p2_w"], np.float32).astype(bf)
    g_w = np.asarray(inputs["g_w"], np.float32).astype(bf)
    pos_w = np.asarray(inputs["pos_w"], np.float32).reshape(9, C).T.copy()
    lepe_w = np.asarray(inputs["lepe_w"], np.float32).reshape(25, C).T.copy()
    cvec = np.zeros((C, 12), np.float32)
    for col, name in ((CV_N1G, "n1_g"), (CV_N1B, "n1_b"), (CV_N2G, "n2_g"),
                      (CV_N2B, "n2_b"), (CV_POSB, "pos_b"), (CV_LEPB, "lepe_b"),
                      (CV_PROJB, "proj_b"), (CV_P2B, "p2_b"), (CV_GB, "g_b")):
        cvec[:, col] = np.asarray(inputs[name], np.float32)
    p1b2 = np.asarray(inputs["p1_b"], np.float32).reshape(8, 128).T.copy()
    iden = np.eye(128, dtype=np.float32)
    in_maps = []
    for core in range(8):
        b, qc = core // 4, core % 4
        mv = np.zeros((128, 4), np.float32)
        mv[:, qc] = 1.0
        in_maps.append({
            "xt": np.ascontiguousarray(x[b].T).astype(bf),
            "qoff": np.array([[Q * qc]], np.uint32),
            "mvec": mv,
            "qkvw": qkv_w, "projw": proj_w, "p1w": p1_w,
            "p2w": p2_w, "gw": g_w,
            "posw": pos_w, "lepw": lepe_w, "cvec": cvec,
            "p1b2": p1b2, "iden": iden,
        })
    return in_maps


def _run(inputs, trace=False):
    nc = _get_nc()
    in_maps = _make_inmaps(inputs)
    res = bass_utils.run_bass_kernel_spmd(nc, in_maps,
                                          core_ids=list(range(8)), trace=trace)
    out = np.zeros((B, N, C), np.float32)
    for core in range(8):
        b, qc = core // 4, core % 4
        out[b, Q * qc:Q * (qc + 1), :] = res.results[core]["y"]
    return out, res


def kernel(**inputs):
    out, _ = _run(inputs, trace=False)
    return out
